# revision 17
# baseline (speedup 1.0000x reference)
"""Trainium2 Bass kernel for nn_ConceptFusionModule.

Math: the reference broadcasts a per-batch (B, D) fused vector over the N
sequence positions *before* rmsnorm + out-projection, so the big
(B, N, D) @ (D, D) matmul is rank-deficient: every row of its output is
identical per batch.  The whole module collapses to

    logits  = textN @ Wc.T                # (B*L, C)
    cw      = softmax(logits, -1)
    crT     = textN_b.T @ cw_b            # (D, C) per batch
    v       = crT.T @ Wv.T                # (B*C, D)
    fused_b = blend@v_b + sig(g)*.3*var_c(v_b)       # (B, D)
    y       = rmsnorm(fused) * nw
    obd     = y @ Wo.T                    # (B, D)
    out     = x + obd[:, None, :]

Wq/Wk cancel (softmax over a single key) and are never loaded.

Sharding: the only O(B*N*D) work is the final broadcast add, which is
data-parallel: each of the 8 cores gets 512 rows of each batch of x.
The small chain (everything above the final add) is replicated on every
core — no collectives.  Host-side work is layout only (slicing /
transposition of inputs); every FLOP of the module runs on device.
"""

import os

import numpy as np

import concourse.bacc as bacc
import concourse.bass as bass
import concourse.mybir as mybir
import concourse.tile as tile
from concourse import masks
from concourse.bass_utils import run_bass_kernel_spmd

F32 = mybir.dt.float32
F32R = mybir.dt.float32r

N_CORES = 8
B, N, L, D, C = 2, 4096, 256, 2048, 6
BL = B * L              # 512 text rows
ROWS = B * N // N_CORES  # 1024 x-rows per core
HALF = ROWS // 2         # 512 rows from each batch
KT = D // 128            # 16 contraction tiles
NCH = D // 512           # 4 free-dim chunks
AX = mybir.AxisListType.X
AF = mybir.ActivationFunctionType

USE_FP32R = os.environ.get("BASS_FP32R", "1") == "1"


def _mm(ap):
    """Matmul-operand dtype: float32r streams one row/cycle (vs 4 for fp32)
    once the moving dim is >=256; tiny matmuls stay plain fp32."""
    return ap.bitcast(F32R) if USE_FP32R else ap


def build_nc(is_surreal: bool) -> bacc.Bacc:
    nc = bacc.Bacc("TRN2", target_bir_lowering=False, debug=False,
                   num_devices=N_CORES)

    x_d = nc.dram_tensor("x_shard", [ROWS, D], F32, kind="ExternalInput")
    tN_d = nc.dram_tensor("textN", [BL, D], F32, kind="ExternalInput")
    tT_d = nc.dram_tensor("textT", [D, BL], F32, kind="ExternalInput")
    wct_d = nc.dram_tensor("WcT", [D, C], F32, kind="ExternalInput")
    wvt_d = nc.dram_tensor("WvT", [D, D], F32, kind="ExternalInput")
    wot_d = nc.dram_tensor("WoT", [D, D], F32, kind="ExternalInput")
    bl_d = nc.dram_tensor("blend", [1, C], F32, kind="ExternalInput")
    sg_d = nc.dram_tensor("sg2", [2, 1], F32, kind="ExternalInput")
    nw_d = nc.dram_tensor("nw2", [2, D], F32, kind="ExternalInput")
    out_d = nc.dram_tensor("out_shard", [ROWS, D], F32, kind="ExternalOutput")

    with tile.TileContext(nc) as tc:
        with (
            tc.tile_pool(name="pc", bufs=1) as pc,
            tc.tile_pool(name="pwrk", bufs=2) as pwrk,
            tc.tile_pool(name="pwv", bufs=3) as pwv,
        ):
            # ---- constant / activations SBUF ----
            blend_sb = pc.tile([1, C], F32)
            sg_sb = pc.tile([2, 1], F32)
            nw_sb = pc.tile([2, D], F32)
            ident = pc.tile([128, 128], F32)
            sel0 = pc.tile([2, 128], F32)
            sel1 = pc.tile([2, 128], F32)
            cwbd = pc.tile([128, 4, 2 * C], F32)    # block-diag cluster weights
            bd4 = pc.tile([2 * C, 4], F32)          # [blend_b0|blend_b1|1_b0|1_b1]
            ones2 = pc.tile([2 * C, 2], F32)
            crT_sb = pc.tile([128, KT, 2 * C], F32)
            yT_sb = pc.tile([128, KT, 2], F32)
            lg_sb = pc.tile([C, BL], F32)
            blendn = pc.tile([1, C], F32)
            v_sb = pc.tile([2 * C, D], F32)
            v2_sb = pc.tile([2 * C, D], F32)
            fused_sb = pc.tile([2, D], F32)
            sq_sb = pc.tile([2, D], F32)
            y_sb = pc.tile([2, D], F32)
            obd_sb = pc.tile([2, D], F32)
            bc0 = pc.tile([128, D], F32)
            bc1 = pc.tile([128, D], F32)
            g3 = pc.tile([2, 1], F32)
            ms = pc.tile([2, 1], F32)
            rs = pc.tile([2, 1], F32)
            eps_t = pc.tile([2, 1], F32)
            m12 = pc.tile([2 * C, 1], F32)
            m12c = pc.tile([2 * C, 1], F32)
            blendn2 = pc.tile([1, 2 * C], F32)
            bmx = pc.tile([1, 1], F32)
            bsum = pc.tile([1, 1], F32)
            brcp = pc.tile([1, 1], F32)

            # ---- text pool: freed after the crT phase to make room for Wo/x ----
            ptext_cm = tc.tile_pool(name="ptext", bufs=1)
            ptext = ptext_cm.__enter__()
            textN = ptext.tile([128, 4, D], F32)    # (l-tile, d) natural
            textT = ptext.tile([128, KT, BL], F32)  # (d-tile, b*l)
            wct = ptext.tile([128, KT, C], F32)

            # ---- phase 0: loads (sync HWDGE ring is FIFO: text -> Wv -> Wo -> x) ----
            nc.sync.dma_start(out=textN[:],
                              in_=tN_d.ap().rearrange("(g p) d -> p g d", p=128))
            nc.sync.dma_start(out=textT[:],
                              in_=tT_d.ap().rearrange("(j p) l -> p j l", p=128))
            nc.sync.dma_start(out=wct[:],
                              in_=wct_d.ap().rearrange("(j p) c -> p j c", p=128))
            nc.sync.dma_start(out=blend_sb[:], in_=bl_d.ap())
            nc.sync.dma_start(out=sg_sb[:], in_=sg_d.ap())
            nc.sync.dma_start(out=nw_sb[:], in_=nw_d.ap())

            # ---- constants built on gpsimd (idle engine) ----
            masks.make_identity(nc, ident[:])
            nc.gpsimd.memset(sel0[:], 0.0)
            nc.gpsimd.memset(sel0[0:1, :], 1.0)
            # sel1 = 1 - sel0 (gpsimd can't memset at partition offset 1)
            nc.vector.tensor_scalar(sel1[:], sel0[:], -1.0, 1.0,
                                    op0=mybir.AluOpType.mult,
                                    op1=mybir.AluOpType.add)
            nc.gpsimd.memset(cwbd[:], 0.0)
            nc.gpsimd.memset(eps_t[:], 1e-6)
            # m12 = [1]*C + [0]*C column; m12c its complement (no partition-
            # offset writes allowed on SBUF, so columns are built via masks)
            nc.gpsimd.memset(m12[:], 0.0)
            nc.gpsimd.memset(m12[0:C, 0:1], 1.0)
            nc.vector.tensor_scalar(m12c[:], m12[:], -1.0, 1.0,
                                    op0=mybir.AluOpType.mult,
                                    op1=mybir.AluOpType.add)
            nc.vector.tensor_copy(ones2[:, 0:1], m12[:])
            nc.vector.tensor_copy(ones2[:, 1:2], m12c[:])
            nc.vector.tensor_copy(bd4[:, 2:3], m12[:])
            nc.vector.tensor_copy(bd4[:, 3:4], m12c[:])

            # ---- logits.T = WcT.T @ textT : (C, B*L), contraction over d ----
            with tc.tile_pool(name="ps_lg", bufs=1, space="PSUM") as ps_lg:
                lg_ps = ps_lg.tile([C, BL], F32)
                for j in range(KT):
                    nc.tensor.matmul(lg_ps[:], _mm(wct[:, j, :]),
                                     _mm(textT[:, j, :]),
                                     start=(j == 0), stop=(j == KT - 1))
                nc.vector.tensor_copy(lg_sb[:], lg_ps[:])

            # ---- softmax over C per token; write into block-diag cwbd ----
            with tc.tile_pool(name="ps_lt", bufs=2, space="PSUM") as ps_lt:
                for t in range(4):
                    lt_ps = ps_lt.tile([128, C], F32)
                    nc.tensor.transpose(lt_ps[:], lg_sb[:, 128 * t:128 * (t + 1)],
                                        ident[0:C, 0:C])
                    nmx = pwrk.tile([128, 1], F32)
                    nc.vector.reduce_max(nmx[:], lt_ps[:], axis=AX, negate=True)
                    e_sb = pwrk.tile([128, C], F32)
                    nc.scalar.activation(e_sb[:], lt_ps[:], AF.Exp, bias=nmx[:])
                    ssum = pwrk.tile([128, 1], F32)
                    nc.vector.reduce_sum(ssum[:], e_sb[:], axis=AX)
                    srcp = pwrk.tile([128, 1], F32)
                    nc.vector.reciprocal(srcp[:], ssum[:])
                    off = 0 if t < 2 else C
                    nc.vector.tensor_scalar_mul(cwbd[:, t, off:off + C],
                                                e_sb[:], srcp[:])

            # ---- crT[d, b*c] = sum_l textN[l, d] * cwbd[l, b*c] ----
            with tc.tile_pool(name="ps_cr", bufs=2, space="PSUM") as ps_cr:
                for j in range(KT):
                    cr_ps = ps_cr.tile([128, 2 * C], F32)
                    for t in range(4):
                        nc.tensor.matmul(cr_ps[:],
                                         textN[:, t, 128 * j:128 * (j + 1)],
                                         cwbd[:, t, :],
                                         start=(t == 0), stop=(t == 3))
                    nc.vector.tensor_copy(crT_sb[:, j, :], cr_ps[:])

            ptext_cm.__exit__(None, None, None)

            # ---- v[b*c, e] = crT.T @ WvT : stream WvT k-tiles from HBM ----
            with tc.tile_pool(name="ps_v", bufs=1, space="PSUM") as ps_v:
                v_ps = [ps_v.tile([2 * C, 512], F32, name=f"v{ch}", tag=f"v{ch}")
                        for ch in range(NCH)]
                for j in range(KT):
                    wvt = pwv.tile([128, D], F32)
                    nc.sync.dma_start(out=wvt[:],
                                      in_=wvt_d[128 * j:128 * (j + 1), :])
                    for ch in range(NCH):
                        nc.tensor.matmul(v_ps[ch][:], _mm(crT_sb[:, j, :]),
                                         _mm(wvt[:, 512 * ch:512 * (ch + 1)]),
                                         start=(j == 0), stop=(j == KT - 1))
                for ch in range(NCH):
                    nc.vector.tensor_copy(v_sb[:, 512 * ch:512 * (ch + 1)],
                                          v_ps[ch][:])

            # ---- blend softmax (tiny) + blendT into bd4 columns ----
            nc.vector.reduce_max(bmx[:], blend_sb[:], axis=AX, negate=True)
            nc.scalar.activation(blendn[:], blend_sb[:], AF.Exp, bias=bmx[:])
            nc.vector.reduce_sum(bsum[:], blendn[:], axis=AX)
            nc.vector.reciprocal(brcp[:], bsum[:])
            nc.vector.tensor_scalar_mul(blendn[:], blendn[:], brcp[:])
            nc.vector.tensor_copy(blendn2[0:1, 0:C], blendn[:])
            nc.vector.tensor_copy(blendn2[0:1, C:2 * C], blendn[:])
            with tc.tile_pool(name="ps_bl", bufs=1, space="PSUM") as ps_bl:
                blt_ps = ps_bl.tile([2 * C, 1], F32)
                nc.tensor.transpose(blt_ps[:], blendn2[:], ident[0:1, 0:1])
                nc.vector.tensor_mul(bd4[:, 0:1], blt_ps[:], m12[:])
                nc.vector.tensor_mul(bd4[:, 1:2], blt_ps[:], m12c[:])

            if is_surreal:
                nc.vector.tensor_mul(v2_sb[:], v_sb[:], v_sb[:])
                # g3 = sigmoid(gate) * 0.3 / (C - 1)
                nc.scalar.activation(g3[:], sg_sb[:], AF.Sigmoid)
                nc.scalar.mul(g3[:], g3[:], 0.3 / (C - 1))

            # ---- fused[b, e] = blend@v + g3*(s2 - s1^2/C) ----
            with (
                tc.tile_pool(name="ps_fl", bufs=2, space="PSUM") as ps_fl,
                tc.tile_pool(name="ps_s2", bufs=2, space="PSUM") as ps_s2,
            ):
                for ch in range(NCH):
                    sl = slice(512 * ch, 512 * (ch + 1))
                    fl_ps = ps_fl.tile([2, 512], F32)
                    nc.tensor.matmul(fl_ps[:], bd4[:, 0:2], v_sb[:, sl],
                                     start=True, stop=True)
                    if is_surreal:
                        s1_ps = ps_fl.tile([2, 512], F32, name=f"s1_{ch}",
                                           tag="s1")
                        nc.tensor.matmul(s1_ps[:], bd4[:, 2:4], v_sb[:, sl],
                                         start=True, stop=True)
                        s2_ps = ps_s2.tile([2, 512], F32)
                        nc.tensor.matmul(s2_ps[:], ones2[:], v2_sb[:, sl],
                                         start=True, stop=True)
                        t1 = pwrk.tile([2, 512], F32)
                        nc.scalar.activation(t1[:], s1_ps[:], AF.Square)
                        t2 = pwrk.tile([2, 512], F32)
                        nc.vector.scalar_tensor_tensor(
                            t2[:], t1[:], -1.0 / C, s2_ps[:],
                            op0=mybir.AluOpType.mult, op1=mybir.AluOpType.add)
                        nc.vector.scalar_tensor_tensor(
                            fused_sb[:, sl], t2[:], g3[0:2, 0:1], fl_ps[:],
                            op0=mybir.AluOpType.mult, op1=mybir.AluOpType.add)
                    else:
                        nc.vector.tensor_copy(fused_sb[:, sl], fl_ps[:])

            # ---- rmsnorm: y = fused * rsqrt(mean(fused^2) + eps) * nw ----
            nc.vector.tensor_mul(sq_sb[:], fused_sb[:], fused_sb[:])
            nc.vector.reduce_sum(ms[:], sq_sb[:], axis=AX)
            nc.scalar.activation(ms[:], ms[:], AF.Sqrt, bias=eps_t[:],
                                 scale=1.0 / D)
            nc.vector.reciprocal(rs[:], ms[:])
            nc.vector.scalar_tensor_tensor(
                y_sb[:], fused_sb[:], rs[0:2, 0:1], nw_sb[:],
                op0=mybir.AluOpType.mult, op1=mybir.AluOpType.mult)

            # ---- yT tiles via PE transpose ----
            with tc.tile_pool(name="ps_yt", bufs=2, space="PSUM") as ps_yt:
                for j in range(KT):
                    yt_ps = ps_yt.tile([128, 2], F32)
                    nc.tensor.transpose(yt_ps[:], y_sb[:, 128 * j:128 * (j + 1)],
                                        ident[0:2, 0:2])
                    nc.vector.tensor_copy(yT_sb[:, j, :], yt_ps[:])

                # ---- obd[b, e'] = yT.T @ WoT : stream WoT k-tiles ----
                with (
                    tc.tile_pool(name="pwo", bufs=5) as pwo,
                    tc.tile_pool(name="ps_ob", bufs=1, space="PSUM") as ps_ob,
                ):
                    ob_ps = [ps_ob.tile([2, 512], F32, name=f"ob{ch}", tag=f"ob{ch}")
                             for ch in range(NCH)]
                    for j in range(KT):
                        wot = pwo.tile([128, D], F32)
                        nc.sync.dma_start(out=wot[:],
                                          in_=wot_d[128 * j:128 * (j + 1), :])
                        for ch in range(NCH):
                            nc.tensor.matmul(ob_ps[ch][:], _mm(yT_sb[:, j, :]),
                                             _mm(wot[:, 512 * ch:512 * (ch + 1)]),
                                             start=(j == 0), stop=(j == KT - 1))
                    for ch in range(NCH):
                        nc.vector.tensor_copy(obd_sb[:, 512 * ch:512 * (ch + 1)],
                                              ob_ps[ch][:])

            # ---- broadcast obd rows to 128 partitions via k=1 matmul ----
            with tc.tile_pool(name="ps_bc", bufs=2, space="PSUM") as ps_bc:
                for sel, bc in ((sel0, bc0), (sel1, bc1)):
                    for ch in range(NCH):
                        sl = slice(512 * ch, 512 * (ch + 1))
                        bc_ps = ps_bc.tile([128, 512], F32)
                        nc.tensor.matmul(bc_ps[:], sel[:], obd_sb[0:2, sl],
                                         start=True, stop=True)
                        nc.vector.tensor_copy(bc[:, sl], bc_ps[:])

            # ---- the only O(N) work: out = x + obd[b] (rows 0..511 are b0) ----
            with tc.tile_pool(name="px", bufs=5) as px:
                for t in range(ROWS // 128):
                    xt = px.tile([128, D], F32)
                    nc.sync.dma_start(out=xt[:], in_=x_d[128 * t:128 * (t + 1), :])
                    bc = bc0 if t < (HALF // 128) else bc1
                    nc.vector.tensor_add(xt[:], xt[:], bc[:])
                    nc.scalar.dma_start(out=out_d[128 * t:128 * (t + 1), :],
                                        in_=xt[:])

    nc.compile()
    return nc


def prep_inputs(x, text_emb, Wc, Wv, Wo, blend_weights, surreal_gate,
                norm_weight):
    """Host-side layout prep (slice/transpose/replicate only)."""
    f = np.float32
    shared = {
        "textN": np.ascontiguousarray(text_emb.reshape(BL, D), dtype=f),
        "textT": np.ascontiguousarray(text_emb.reshape(BL, D).T, dtype=f),
        "WcT": np.ascontiguousarray(Wc.T, dtype=f),
        "WvT": np.ascontiguousarray(Wv.T, dtype=f),
        "WoT": np.ascontiguousarray(Wo.T, dtype=f),
        "blend": np.ascontiguousarray(blend_weights.reshape(1, C), dtype=f),
        "sg2": np.broadcast_to(np.asarray(surreal_gate, f).reshape(1, 1),
                               (2, 1)).copy(),
        "nw2": np.broadcast_to(np.asarray(norm_weight, f), (2, D)).copy(),
    }
    in_maps = []
    for k in range(N_CORES):
        xs = np.concatenate(
            [x[0, HALF * k:HALF * (k + 1), :], x[1, HALF * k:HALF * (k + 1), :]],
            axis=0).astype(f)
        in_maps.append({"x_shard": np.ascontiguousarray(xs), **shared})
    return in_maps


_CACHE = {}


def kernel(x, text_emb, Wc, Wq, Wk, Wv, Wo, blend_weights, surreal_gate,
           norm_weight, is_surreal, _collect=None):
    surreal = bool(int(np.asarray(is_surreal)))
    key = ("nc", surreal)
    if key not in _CACHE:
        _CACHE[key] = build_nc(surreal)
    nc = _CACHE[key]

    in_maps = prep_inputs(x, text_emb, Wc, Wv, Wo, blend_weights,
                          surreal_gate, norm_weight)
    res = run_bass_kernel_spmd(
        nc, in_maps, core_ids=list(range(N_CORES)),
        trace=os.environ.get("KERNEL_TRACE", "0") == "1",
    )
    if _collect is not None:
        _collect.append(res)

    out = np.empty((B, N, D), np.float32)
    for k in range(N_CORES):
        shard = res.results[k]["out_shard"]
        out[0, HALF * k:HALF * (k + 1), :] = shard[:HALF]
        out[1, HALF * k:HALF * (k + 1), :] = shard[HALF:]
    return out


# revision 18
# speedup vs baseline: 1.2056x; 1.2056x over previous
"""Trainium2 Bass kernel for nn_ConceptFusionModule.

Math: the reference broadcasts a per-batch (B, D) fused vector over the N
sequence positions *before* rmsnorm + out-projection, so the big
(B, N, D) @ (D, D) matmul is rank-deficient: every row of its output is
identical per batch.  The whole module collapses to

    logits  = textN @ Wc.T                # (B*L, C)
    cw      = softmax(logits, -1)
    crT     = textN_b.T @ cw_b            # (D, C) per batch
    v       = crT.T @ Wv.T                # (B*C, D)
    fused_b = blend@v_b + sig(g)*.3*var_c(v_b)       # (B, D)
    y       = rmsnorm(fused) * nw
    obd     = y @ Wo.T                    # (B, D)
    out     = x + obd[:, None, :]

Wq/Wk cancel (softmax over a single key) and are never loaded.

Sharding: the only O(B*N*D) work is the final broadcast add, which is
data-parallel: each of the 8 cores gets 512 rows of each batch of x.
The small chain (everything above the final add) is replicated on every
core — no collectives.  Host-side work is layout only (slicing /
transposition of inputs); every FLOP of the module runs on device.
"""

import os

import numpy as np

import concourse.bacc as bacc
import concourse.bass as bass
import concourse.mybir as mybir
import concourse.tile as tile
from concourse import masks
from concourse.bass_utils import run_bass_kernel_spmd

F32 = mybir.dt.float32
F32R = mybir.dt.float32r

N_CORES = 8
B, N, L, D, C = 2, 4096, 256, 2048, 6
BL = B * L              # 512 text rows
ROWS = B * N // N_CORES  # 1024 x-rows per core
HALF = ROWS // 2         # 512 rows from each batch
KT = D // 128            # 16 contraction tiles
NCH = D // 512           # 4 free-dim chunks
AX = mybir.AxisListType.X
AF = mybir.ActivationFunctionType

USE_FP32R = os.environ.get("BASS_FP32R", "1") == "1"


# float32r streams one row/cycle (vs 4 for fp32) once the moving dim is
# >=256.  The verifier requires fp32r matmul operands to be *produced* as
# fp32r, so the big matmul-input tiles are allocated in that dtype and the
# (bit-identical) DRAM source APs are bitcast for the load.
MMDT = F32R if USE_FP32R else F32


def _mm(ap):
    return ap


def build_nc(is_surreal: bool) -> bacc.Bacc:
    nc = bacc.Bacc("TRN2", target_bir_lowering=False, debug=False,
                   num_devices=N_CORES)

    x_d = nc.dram_tensor("x_shard", [ROWS, D], F32, kind="ExternalInput")
    tN_d = nc.dram_tensor("textN", [BL, D], F32, kind="ExternalInput")
    tT_d = nc.dram_tensor("textT", [D, BL], F32, kind="ExternalInput")
    wct_d = nc.dram_tensor("WcT", [D, C], F32, kind="ExternalInput")
    wvt_d = nc.dram_tensor("WvT", [D, D], F32, kind="ExternalInput")
    wot_d = nc.dram_tensor("WoT", [D, D], F32, kind="ExternalInput")
    bl_d = nc.dram_tensor("blend", [1, C], F32, kind="ExternalInput")
    sg_d = nc.dram_tensor("sg2", [2, 1], F32, kind="ExternalInput")
    nw_d = nc.dram_tensor("nw2", [2, D], F32, kind="ExternalInput")
    out_d = nc.dram_tensor("out_shard", [ROWS, D], F32, kind="ExternalOutput")

    with tile.TileContext(nc) as tc:
        with (
            tc.tile_pool(name="pc", bufs=1) as pc,
            tc.tile_pool(name="pwrk", bufs=2) as pwrk,
            tc.tile_pool(name="pwv", bufs=3) as pwv,
        ):
            # ---- constant / activations SBUF ----
            blend_sb = pc.tile([1, C], F32)
            sg_sb = pc.tile([2, 1], F32)
            nw_sb = pc.tile([2, D], F32)
            ident = pc.tile([128, 128], F32)
            sel0 = pc.tile([2, 128], F32)
            sel1 = pc.tile([2, 128], F32)
            cwbd = pc.tile([128, 4, 2 * C], F32)    # block-diag cluster weights
            bd4 = pc.tile([2 * C, 4], F32)          # [blend_b0|blend_b1|1_b0|1_b1]
            ones2 = pc.tile([2 * C, 2], F32)
            crT_sb = pc.tile([128, KT, 2 * C], MMDT)
            yT_sb = pc.tile([128, KT, 2], MMDT)
            lg_sb = pc.tile([C, BL], F32)
            blendn = pc.tile([1, C], F32)
            v_sb = pc.tile([2 * C, D], F32)
            v2_sb = pc.tile([2 * C, D], F32)
            fused_sb = pc.tile([2, D], F32)
            sq_sb = pc.tile([2, D], F32)
            y_sb = pc.tile([2, D], F32)
            obd_sb = pc.tile([2, D], F32)
            bc0 = pc.tile([128, D], F32)
            bc1 = pc.tile([128, D], F32)
            g3 = pc.tile([2, 1], F32)
            ms = pc.tile([2, 1], F32)
            rs = pc.tile([2, 1], F32)
            eps_t = pc.tile([2, 1], F32)
            m12 = pc.tile([2 * C, 1], F32)
            m12c = pc.tile([2 * C, 1], F32)
            blendn2 = pc.tile([1, 2 * C], F32)
            bmx = pc.tile([1, 1], F32)
            bsum = pc.tile([1, 1], F32)
            brcp = pc.tile([1, 1], F32)

            # ---- text pool: freed after the crT phase to make room for Wo/x ----
            ptext_cm = tc.tile_pool(name="ptext", bufs=1)
            ptext = ptext_cm.__enter__()
            textN = ptext.tile([128, 4, D], F32)    # (l-tile, d) natural
            textT = ptext.tile([128, KT, BL], MMDT)  # (d-tile, b*l)
            wct = ptext.tile([128, KT, C], MMDT)

            # ---- phase 0: loads (sync HWDGE ring is FIFO: text -> Wv -> Wo -> x) ----
            nc.sync.dma_start(out=textN[:],
                              in_=tN_d.ap().rearrange("(g p) d -> p g d", p=128))
            nc.sync.dma_start(out=textT[:],
                              in_=tT_d.ap().rearrange("(j p) l -> p j l", p=128).bitcast(MMDT))
            nc.sync.dma_start(out=wct[:],
                              in_=wct_d.ap().rearrange("(j p) c -> p j c", p=128).bitcast(MMDT))
            nc.sync.dma_start(out=blend_sb[:], in_=bl_d.ap())
            nc.sync.dma_start(out=sg_sb[:], in_=sg_d.ap())
            nc.sync.dma_start(out=nw_sb[:], in_=nw_d.ap())

            # ---- constants built on gpsimd (idle engine) ----
            masks.make_identity(nc, ident[:])
            nc.gpsimd.memset(sel0[:], 0.0)
            nc.gpsimd.memset(sel0[0:1, :], 1.0)
            # sel1 = 1 - sel0 (gpsimd can't memset at partition offset 1)
            nc.vector.tensor_scalar(sel1[:], sel0[:], -1.0, 1.0,
                                    op0=mybir.AluOpType.mult,
                                    op1=mybir.AluOpType.add)
            nc.gpsimd.memset(cwbd[:], 0.0)
            nc.gpsimd.memset(eps_t[:], 1e-6)
            # m12 = [1]*C + [0]*C column; m12c its complement (no partition-
            # offset writes allowed on SBUF, so columns are built via masks)
            nc.gpsimd.memset(m12[:], 0.0)
            nc.gpsimd.memset(m12[0:C, 0:1], 1.0)
            nc.vector.tensor_scalar(m12c[:], m12[:], -1.0, 1.0,
                                    op0=mybir.AluOpType.mult,
                                    op1=mybir.AluOpType.add)
            nc.vector.tensor_copy(ones2[:, 0:1], m12[:])
            nc.vector.tensor_copy(ones2[:, 1:2], m12c[:])
            nc.vector.tensor_copy(bd4[:, 2:3], m12[:])
            nc.vector.tensor_copy(bd4[:, 3:4], m12c[:])

            # ---- logits.T = WcT.T @ textT : (C, B*L), contraction over d ----
            with tc.tile_pool(name="ps_lg", bufs=1, space="PSUM") as ps_lg:
                lg_ps = ps_lg.tile([C, BL], F32)
                for j in range(KT):
                    nc.tensor.matmul(lg_ps[:], _mm(wct[:, j, :]),
                                     _mm(textT[:, j, :]),
                                     start=(j == 0), stop=(j == KT - 1))
                nc.vector.tensor_copy(lg_sb[:], lg_ps[:])

            # ---- softmax over C per token; write into block-diag cwbd ----
            with tc.tile_pool(name="ps_lt", bufs=2, space="PSUM") as ps_lt:
                for t in range(4):
                    lt_ps = ps_lt.tile([128, C], F32)
                    nc.tensor.transpose(lt_ps[:], lg_sb[:, 128 * t:128 * (t + 1)],
                                        ident[0:C, 0:C])
                    nmx = pwrk.tile([128, 1], F32)
                    nc.vector.reduce_max(nmx[:], lt_ps[:], axis=AX, negate=True)
                    e_sb = pwrk.tile([128, C], F32)
                    nc.scalar.activation(e_sb[:], lt_ps[:], AF.Exp, bias=nmx[:])
                    ssum = pwrk.tile([128, 1], F32)
                    nc.vector.reduce_sum(ssum[:], e_sb[:], axis=AX)
                    srcp = pwrk.tile([128, 1], F32)
                    nc.vector.reciprocal(srcp[:], ssum[:])
                    off = 0 if t < 2 else C
                    nc.vector.tensor_scalar_mul(cwbd[:, t, off:off + C],
                                                e_sb[:], srcp[:])

            # ---- crT[d, b*c] = sum_l textN[l, d] * cwbd[l, b*c] ----
            with tc.tile_pool(name="ps_cr", bufs=2, space="PSUM") as ps_cr:
                for j in range(KT):
                    cr_ps = ps_cr.tile([128, 2 * C], F32)
                    for t in range(4):
                        nc.tensor.matmul(cr_ps[:],
                                         textN[:, t, 128 * j:128 * (j + 1)],
                                         cwbd[:, t, :],
                                         start=(t == 0), stop=(t == 3))
                    nc.vector.tensor_copy(crT_sb[:, j, :], cr_ps[:])

            ptext_cm.__exit__(None, None, None)

            # ---- v[b*c, e] = crT.T @ WvT : stream WvT k-tiles from HBM ----
            with tc.tile_pool(name="ps_v", bufs=1, space="PSUM") as ps_v:
                v_ps = [ps_v.tile([2 * C, 512], F32, name=f"v{ch}", tag=f"v{ch}")
                        for ch in range(NCH)]
                for j in range(KT):
                    wvt = pwv.tile([128, D], MMDT)
                    nc.sync.dma_start(out=wvt[:],
                                      in_=wvt_d[128 * j:128 * (j + 1), :].bitcast(MMDT))
                    for ch in range(NCH):
                        nc.tensor.matmul(v_ps[ch][:], _mm(crT_sb[:, j, :]),
                                         _mm(wvt[:, 512 * ch:512 * (ch + 1)]),
                                         start=(j == 0), stop=(j == KT - 1))
                for ch in range(NCH):
                    nc.vector.tensor_copy(v_sb[:, 512 * ch:512 * (ch + 1)],
                                          v_ps[ch][:])

            # ---- blend softmax (tiny) + blendT into bd4 columns ----
            nc.vector.reduce_max(bmx[:], blend_sb[:], axis=AX, negate=True)
            nc.scalar.activation(blendn[:], blend_sb[:], AF.Exp, bias=bmx[:])
            nc.vector.reduce_sum(bsum[:], blendn[:], axis=AX)
            nc.vector.reciprocal(brcp[:], bsum[:])
            nc.vector.tensor_scalar_mul(blendn[:], blendn[:], brcp[:])
            nc.vector.tensor_copy(blendn2[0:1, 0:C], blendn[:])
            nc.vector.tensor_copy(blendn2[0:1, C:2 * C], blendn[:])
            with tc.tile_pool(name="ps_bl", bufs=1, space="PSUM") as ps_bl:
                blt_ps = ps_bl.tile([2 * C, 1], F32)
                nc.tensor.transpose(blt_ps[:], blendn2[:], ident[0:1, 0:1])
                nc.vector.tensor_mul(bd4[:, 0:1], blt_ps[:], m12[:])
                nc.vector.tensor_mul(bd4[:, 1:2], blt_ps[:], m12c[:])

            if is_surreal:
                nc.vector.tensor_mul(v2_sb[:], v_sb[:], v_sb[:])
                # g3 = sigmoid(gate) * 0.3 / (C - 1)
                nc.scalar.activation(g3[:], sg_sb[:], AF.Sigmoid)
                nc.scalar.mul(g3[:], g3[:], 0.3 / (C - 1))

            # ---- fused[b, e] = blend@v + g3*(s2 - s1^2/C) ----
            with (
                tc.tile_pool(name="ps_fl", bufs=2, space="PSUM") as ps_fl,
                tc.tile_pool(name="ps_s2", bufs=2, space="PSUM") as ps_s2,
            ):
                for ch in range(NCH):
                    sl = slice(512 * ch, 512 * (ch + 1))
                    fl_ps = ps_fl.tile([2, 512], F32)
                    nc.tensor.matmul(fl_ps[:], bd4[:, 0:2], v_sb[:, sl],
                                     start=True, stop=True)
                    if is_surreal:
                        s1_ps = ps_fl.tile([2, 512], F32, name=f"s1_{ch}",
                                           tag="s1")
                        nc.tensor.matmul(s1_ps[:], bd4[:, 2:4], v_sb[:, sl],
                                         start=True, stop=True)
                        s2_ps = ps_s2.tile([2, 512], F32)
                        nc.tensor.matmul(s2_ps[:], ones2[:], v2_sb[:, sl],
                                         start=True, stop=True)
                        t1 = pwrk.tile([2, 512], F32)
                        nc.scalar.activation(t1[:], s1_ps[:], AF.Square)
                        t2 = pwrk.tile([2, 512], F32)
                        nc.vector.scalar_tensor_tensor(
                            t2[:], t1[:], -1.0 / C, s2_ps[:],
                            op0=mybir.AluOpType.mult, op1=mybir.AluOpType.add)
                        nc.vector.scalar_tensor_tensor(
                            fused_sb[:, sl], t2[:], g3[0:2, 0:1], fl_ps[:],
                            op0=mybir.AluOpType.mult, op1=mybir.AluOpType.add)
                    else:
                        nc.vector.tensor_copy(fused_sb[:, sl], fl_ps[:])

            # ---- rmsnorm: y = fused * rsqrt(mean(fused^2) + eps) * nw ----
            nc.vector.tensor_mul(sq_sb[:], fused_sb[:], fused_sb[:])
            nc.vector.reduce_sum(ms[:], sq_sb[:], axis=AX)
            nc.scalar.activation(ms[:], ms[:], AF.Sqrt, bias=eps_t[:],
                                 scale=1.0 / D)
            nc.vector.reciprocal(rs[:], ms[:])
            nc.vector.scalar_tensor_tensor(
                y_sb[:], fused_sb[:], rs[0:2, 0:1], nw_sb[:],
                op0=mybir.AluOpType.mult, op1=mybir.AluOpType.mult)

            # ---- yT tiles via PE transpose ----
            with tc.tile_pool(name="ps_yt", bufs=2, space="PSUM") as ps_yt:
                for j in range(KT):
                    yt_ps = ps_yt.tile([128, 2], F32)
                    nc.tensor.transpose(yt_ps[:], y_sb[:, 128 * j:128 * (j + 1)],
                                        ident[0:2, 0:2])
                    nc.vector.tensor_copy(yT_sb[:, j, :], yt_ps[:])

                # ---- obd[b, e'] = yT.T @ WoT : stream WoT k-tiles ----
                with (
                    tc.tile_pool(name="pwo", bufs=5) as pwo,
                    tc.tile_pool(name="ps_ob", bufs=1, space="PSUM") as ps_ob,
                ):
                    ob_ps = [ps_ob.tile([2, 512], F32, name=f"ob{ch}", tag=f"ob{ch}")
                             for ch in range(NCH)]
                    for j in range(KT):
                        wot = pwo.tile([128, D], MMDT)
                        nc.sync.dma_start(out=wot[:],
                                          in_=wot_d[128 * j:128 * (j + 1), :].bitcast(MMDT))
                        for ch in range(NCH):
                            nc.tensor.matmul(ob_ps[ch][:], _mm(yT_sb[:, j, :]),
                                             _mm(wot[:, 512 * ch:512 * (ch + 1)]),
                                             start=(j == 0), stop=(j == KT - 1))
                    for ch in range(NCH):
                        nc.vector.tensor_copy(obd_sb[:, 512 * ch:512 * (ch + 1)],
                                              ob_ps[ch][:])

            # ---- broadcast obd rows to 128 partitions via k=1 matmul ----
            with tc.tile_pool(name="ps_bc", bufs=2, space="PSUM") as ps_bc:
                for sel, bc in ((sel0, bc0), (sel1, bc1)):
                    for ch in range(NCH):
                        sl = slice(512 * ch, 512 * (ch + 1))
                        bc_ps = ps_bc.tile([128, 512], F32)
                        nc.tensor.matmul(bc_ps[:], sel[:], obd_sb[0:2, sl],
                                         start=True, stop=True)
                        nc.vector.tensor_copy(bc[:, sl], bc_ps[:])

            # ---- the only O(N) work: out = x + obd[b] (rows 0..511 are b0) ----
            with tc.tile_pool(name="px", bufs=5) as px:
                for t in range(ROWS // 128):
                    xt = px.tile([128, D], F32)
                    nc.sync.dma_start(out=xt[:], in_=x_d[128 * t:128 * (t + 1), :])
                    bc = bc0 if t < (HALF // 128) else bc1
                    nc.vector.tensor_add(xt[:], xt[:], bc[:])
                    nc.scalar.dma_start(out=out_d[128 * t:128 * (t + 1), :],
                                        in_=xt[:])

    nc.compile()
    return nc


def prep_inputs(x, text_emb, Wc, Wv, Wo, blend_weights, surreal_gate,
                norm_weight):
    """Host-side layout prep (slice/transpose/replicate only)."""
    f = np.float32
    shared = {
        "textN": np.ascontiguousarray(text_emb.reshape(BL, D), dtype=f),
        "textT": np.ascontiguousarray(text_emb.reshape(BL, D).T, dtype=f),
        "WcT": np.ascontiguousarray(Wc.T, dtype=f),
        "WvT": np.ascontiguousarray(Wv.T, dtype=f),
        "WoT": np.ascontiguousarray(Wo.T, dtype=f),
        "blend": np.ascontiguousarray(blend_weights.reshape(1, C), dtype=f),
        "sg2": np.broadcast_to(np.asarray(surreal_gate, f).reshape(1, 1),
                               (2, 1)).copy(),
        "nw2": np.broadcast_to(np.asarray(norm_weight, f), (2, D)).copy(),
    }
    in_maps = []
    for k in range(N_CORES):
        xs = np.concatenate(
            [x[0, HALF * k:HALF * (k + 1), :], x[1, HALF * k:HALF * (k + 1), :]],
            axis=0).astype(f)
        in_maps.append({"x_shard": np.ascontiguousarray(xs), **shared})
    return in_maps


_CACHE = {}


def kernel(x, text_emb, Wc, Wq, Wk, Wv, Wo, blend_weights, surreal_gate,
           norm_weight, is_surreal, _collect=None):
    surreal = bool(int(np.asarray(is_surreal)))
    key = ("nc", surreal)
    if key not in _CACHE:
        _CACHE[key] = build_nc(surreal)
    nc = _CACHE[key]

    in_maps = prep_inputs(x, text_emb, Wc, Wv, Wo, blend_weights,
                          surreal_gate, norm_weight)
    res = run_bass_kernel_spmd(
        nc, in_maps, core_ids=list(range(N_CORES)),
        trace=os.environ.get("KERNEL_TRACE", "0") == "1",
    )
    if _collect is not None:
        _collect.append(res)

    out = np.empty((B, N, D), np.float32)
    for k in range(N_CORES):
        shard = res.results[k]["out_shard"]
        out[0, HALF * k:HALF * (k + 1), :] = shard[:HALF]
        out[1, HALF * k:HALF * (k + 1), :] = shard[HALF:]
    return out
